# revision 12
# baseline (speedup 1.0000x reference)
"""DecoderRNN (LSTM decoder + vocab projection) Trainium2 kernel, v3.

Strategy (8 NeuronCores, no collectives):
  - LSTM recurrence (T=64 steps over [B=32, H=512]) replicated on all 8 cores;
    output projection vocab-sharded (core i -> logits[:, :, 4000i:4000(i+1)]).
  - Embedding lookup + input transposes are host-side input marshalling: the
    host ships xT = [E, T*B] bf16 (t-major tokens, features at t=0).
  - Gate layout ("gates2"): PSUM tile [128, 512] with partition = 32*jh + b
    (jh = H-chunk 0..3, b = batch) and free = 128*c + hh with gate order
    c in {f, g, i, o}. The weight matrix is host-permuted so col-group-packed
    matmuls (stationary x^T / h^T [128, 32], streaming W [128, 512], 4
    concurrent col groups) produce this layout directly. All elementwise ops
    then run on [128, 128] tiles (full partition width, short free dim).
  - Gate bias enters via an ACT-engine copy into the gates PSUM bank before
    the matmuls accumulate onto it with start=False. PSUM has_written bits
    persist once set, so a one-time dummy matmul per bank (start=True over
    the full tile) makes every later start=False matmul accumulate instead
    of overwrite. This removes 64 bias-matmul rounds from the PE stream.
  - Sigmoid split (f,g,i then o) so the c' chain starts one op earlier; g is
    host-prescaled by 2 so sigmoid(2z) = (tanh(z)+1)/2 covers tanh. w1 = f*c
    runs on GpSimd (SBUF-only operands) in parallel with u on DVE.
  - h [128, 128] is PE-transposed (one 128x128 transpose) and scattered into
    the hsT archive [128, 4*T*B] (col 2048j + 32t + b), which serves as lhsT
    for both the next h-matmul and the logits matmuls.
  - Logits: per (mt, vn) chunk, 4 accumulating matmuls into PSUM [128, 500];
    DVE adds the (host-replicated) output bias while copying PSUM -> SBUF;
    SP DMA writes straight to the output slice. Chunk B is split around the
    transpose so the PE FIFO stays busy during the hsT copy.
"""

import sys

sys.path.insert(0, "/opt/trn_rl_repo")

import numpy as np
import ml_dtypes

import concourse.bass as bass
import concourse.bacc as bacc
import concourse.tile as tile
import concourse.mybir as mybir
from concourse.bass_utils import run_bass_kernel_spmd

dt = mybir.dt
AF = mybir.ActivationFunctionType
ALU = mybir.AluOpType
BF16 = dt.bfloat16
F32 = dt.float32
bfnp = ml_dtypes.bfloat16

B, T, E, H, V = 32, 64, 512, 512, 32000
NCORES = 8
VC = V // NCORES          # 4000 vocab per core
VN = 500                  # logits n-chunk (8 chunks of 500 = 4000)
NVC = VC // VN            # 8
NT = (T * B) // 128       # 16 token tiles of 128
P = 128

_cached = {}


def _build_nc():
    key = "nc"
    if key in _cached:
        return _cached[key]

    nc = bacc.Bacc("TRN2", target_bir_lowering=False, debug=False)

    # ---- per-core inputs
    xT_d = nc.dram_tensor("xT", [E, T * B], BF16, kind="ExternalInput")
    wt_d = nc.dram_tensor("wt", [E + H, 4 * H], BF16, kind="ExternalInput")
    biasg_d = nc.dram_tensor("biasg", [P, H], F32, kind="ExternalInput")
    ident_d = nc.dram_tensor("ident", [P, P], BF16, kind="ExternalInput")
    wot_d = nc.dram_tensor("wot", [H, VC], BF16, kind="ExternalInput")
    bout_d = nc.dram_tensor("bout", [P, VC], F32, kind="ExternalInput")
    out_d = nc.dram_tensor("out", [T * B, VC], F32, kind="ExternalOutput")

    with tile.TileContext(nc) as tc:
        with (
            tc.tile_pool(name="const", bufs=1) as const,
            tc.tile_pool(name="arch", bufs=1) as arch_p,
            tc.tile_pool(name="sig", bufs=2) as sigp,
            tc.tile_pool(name="work", bufs=3) as work,
            tc.tile_pool(name="lo_out", bufs=3) as lop,
            tc.tile_pool(name="ps_gates", bufs=2, space="PSUM") as ps_g,
            tc.tile_pool(name="ps_tr", bufs=2, space="PSUM") as ps_t,
            tc.tile_pool(name="ps_lo", bufs=3, space="PSUM") as ps_l,
        ):
            # ---------- weights into SBUF ----------
            # urgent loads (needed by t=0/1) on the SP queue; bulk loads that
            # are only needed from t>=1 (h-weights tail, logits operands) go
            # on the otherwise-idle GpSimd software-DGE queue.
            ones_sb = const.tile([1, P], BF16, tag="ones")
            nc.vector.memset(ones_sb[:], 1.0)
            dz_sb = const.tile([1, H], BF16, tag="dz")
            nc.vector.memset(dz_sb[:], 0.0)

            biasg_sb = const.tile([P, H], F32, tag="biasg")
            nc.scalar.dma_start(biasg_sb[:], biasg_d[:])
            ident_sb = const.tile([P, P], BF16, tag="ident")
            nc.scalar.dma_start(ident_sb[:], ident_d[:])

            HEAD = 512           # first 16 steps' tokens
            w_kt = []
            for kt in range(8):
                wt_t = const.tile([P, 4 * H], BF16, tag=f"w{kt}", name=f"w{kt}")
                w_kt.append(wt_t)
            xT_kt = []
            for j in range(4):
                xt_t = const.tile([P, T * B], BF16, tag=f"xT{j}", name=f"xT{j}")
                xT_kt.append(xt_t)
            # x-weights first, split in halves (jh-group region deps), then
            # the x head tokens, then h-weights.
            for kt in range(4):
                nc.sync.dma_start(w_kt[kt][:, 0:1024], wt_d[P * kt : P * (kt + 1), 0:1024])
            for j in range(4):
                nc.sync.dma_start(xT_kt[j][:, 0:HEAD], xT_d[P * j : P * (j + 1), 0:HEAD])
            for kt in range(4):
                nc.sync.dma_start(w_kt[kt][:, 1024:], wt_d[P * kt : P * (kt + 1), 1024:])
            for kt in range(4, 8):
                nc.sync.dma_start(w_kt[kt][:], wt_d[P * kt : P * (kt + 1), :])

            for j in range(4):
                nc.gpsimd.dma_start(
                    xT_kt[j][:, HEAD:], xT_d[P * j : P * (j + 1), HEAD:]
                )

            wot_kt = []
            for j in range(4):
                wo_t = const.tile([P, VC], BF16, tag=f"wot{j}")
                nc.scalar.dma_start(wo_t[:], wot_d[P * j : P * (j + 1), :])
                wot_kt.append(wo_t)
            bout_sb = const.tile([P, VC], F32, tag="bout")
            nc.scalar.dma_start(bout_sb[:], bout_d[:])

            # hsT archive: [128, 4*T*B] bf16; col 2048*j + 32*t + b holds
            # h_t[b, 128j + hh] at partition hh (j = H-chunk).
            arch = arch_p.tile([P, 4 * T * B], BF16, tag="hsT")
            arch_v = arch[:].rearrange("p (j t b) -> p j t b", j=4, t=T)

            # cell state: [128, 128] fp32, partition 32jh+b, free hh
            c2 = const.tile([P, P], F32, tag="c2")
            nc.vector.memset(c2[:], 0.0)

            def emit_dummy(g2):
                """one-time per-PSUM-bank: set has_written over the full tile
                so later start=False matmuls accumulate."""
                nc.tensor.matmul(
                    g2[:],
                    lhsT=ones_sb[0:1, :],
                    rhs=dz_sb[0:1, :],
                    start=True,
                    stop=True,
                    skip_group_check=True,
                )

            def emit_bias(g2):
                """ACT writes the gate bias into the PSUM bank; subsequent
                start=False matmuls accumulate on top."""
                nc.scalar.copy(g2[:], biasg_sb[:])

            def emit_gates_x(t, g2):
                """x rounds for step t (start=False onto pre-written bias)."""
                for kt in range(4):
                    lhsT = xT_kt[kt][:, B * t : B * (t + 1)]
                    for jh in range(4):
                        nc.tensor.matmul(
                            g2[32 * jh : 32 * (jh + 1), :],
                            lhsT=lhsT,
                            rhs=w_kt[kt][:, 512 * jh : 512 * (jh + 1)],
                            start=False,
                            stop=(t == 0 and kt == 3),
                            tile_position=(0, 32 * jh),
                            skip_group_check=True,
                        )

            def emit_gates_h(t, g2):
                """h rounds for step t (reads arch cols of step t-1)."""
                for kt in range(4):
                    base = 2048 * kt + B * (t - 1)
                    lhsT = arch[:, base : base + B]
                    for jh in range(4):
                        nc.tensor.matmul(
                            g2[32 * jh : 32 * (jh + 1), :],
                            lhsT=lhsT,
                            rhs=w_kt[4 + kt][:, 512 * jh : 512 * (jh + 1)],
                            start=False,
                            stop=(kt == 3),
                            tile_position=(0, 32 * jh),
                            skip_group_check=True,
                        )

            def emit_logits_mm(mt, vn, js):
                for j in js:
                    nc.tensor.matmul(
                        lo_ps[mt, vn][:],
                        lhsT=arch[:, 2048 * j + P * mt : 2048 * j + P * (mt + 1)],
                        rhs=wot_kt[j][:, VN * vn : VN * (vn + 1)],
                        start=(j == 0),
                        stop=(j == 3),
                    )

            lo_ps = {}

            def emit_logits_head(mt, vn, js):
                lo_ps[mt, vn] = ps_l.tile([P, VN], F32, tag="lo", name=f"lo_{mt}_{vn}")
                emit_logits_mm(mt, vn, js)

            def emit_logits_tail(mt, vn, js):
                emit_logits_mm(mt, vn, js)
                lo_sb = lop.tile([P, VN], F32, tag="lo_sb")
                nc.vector.tensor_tensor(
                    out=lo_sb[:], in0=lo_ps[mt, vn][:],
                    in1=bout_sb[:, VN * vn : VN * (vn + 1)], op=ALU.add,
                )
                nc.sync.dma_start(
                    out_d[P * mt : P * (mt + 1), VN * vn : VN * (vn + 1)],
                    lo_sb[:],
                )
                del lo_ps[mt, vn]

            # ---------- the 64 recurrence steps ----------
            g2_cur = ps_g.tile([P, H], F32, tag="g2")
            emit_dummy(g2_cur)
            emit_bias(g2_cur)
            emit_gates_x(0, g2_cur)

            for t in range(T):
                # next step's PSUM bank: bias pre-write (ACT queue head, runs
                # while this step's h-rounds stream on the PE)
                if t + 1 < T:
                    g2_next = ps_g.tile([P, H], F32, tag="g2")
                    if t == 0:
                        emit_dummy(g2_next)
                    emit_bias(g2_next)

                if t > 0:
                    emit_gates_h(t, g2_cur)

                # ---- elementwise: free-dim gate chunks f|g|i|o of 128 ----
                sig = sigp.tile([P, H], F32, tag="sig")
                nc.scalar.activation(sig[:, 0:384], g2_cur[:, 0:384], AF.Sigmoid)
                nc.scalar.activation(sig[:, 384:512], g2_cur[:, 384:512], AF.Sigmoid)
                # u = (sg - 0.5) * si = 0.5 * g * i   (DVE)
                u_t = work.tile([P, P], F32, tag="u")
                nc.vector.scalar_tensor_tensor(
                    out=u_t[:], in0=sig[:, 128:256], scalar=0.5,
                    in1=sig[:, 256:384], op0=ALU.subtract, op1=ALU.mult,
                )
                # w1 = f * c
                w1 = work.tile([P, P], F32, tag="w1")
                nc.vector.scalar_tensor_tensor(
                    out=w1[:], in0=sig[:, 0:128], scalar=0.0,
                    in1=c2[:], op0=ALU.add, op1=ALU.mult,
                )
                # c' = 2*u + w1
                nc.vector.scalar_tensor_tensor(
                    out=c2[:], in0=u_t[:], scalar=2.0,
                    in1=w1[:], op0=ALU.mult, op1=ALU.add,
                )
                tc_t = work.tile([P, P], F32, tag="tc")
                nc.scalar.activation(tc_t[:], c2[:], AF.Tanh)
                # h = o * tanh(c')  (bf16)
                h2 = work.tile([P, P], BF16, tag="h2")
                nc.vector.scalar_tensor_tensor(
                    out=h2[:], in0=sig[:, 384:512], scalar=0.0,
                    in1=tc_t[:], op0=ALU.add, op1=ALU.mult,
                )

                # ---- prefill next step's x rounds; logits fill the rest ----
                if t + 1 < T:
                    emit_gates_x(t + 1, g2_next)

                if t >= 4 and t >= 14:
                    mt, k = t // 4 - 1, t % 4
                    emit_logits_head(mt, 2 * k, [0, 1, 2, 3])
                    emit_logits_tail(mt, 2 * k, [])
                    emit_logits_head(mt, 2 * k + 1, [0, 1])

                # ---- transpose h into the archive ----
                htr = ps_t.tile([P, P], BF16, tag="tr")
                nc.tensor.transpose(htr[:], in_=h2[:], identity=ident_sb[:])
                nc.vector.tensor_copy(
                    arch_v[:, :, t, :],
                    htr[:].rearrange("p (j b) -> p j b", j=4),
                )

                if t >= 4:
                    if t < 14:
                        mt, k = t // 4 - 1, t % 4
                        emit_logits_head(mt, 2 * k, [0, 1, 2, 3])
                        emit_logits_tail(mt, 2 * k, [])
                        emit_logits_head(mt, 2 * k + 1, [0, 1])
                    emit_logits_tail(mt, 2 * k + 1, [2, 3])

                if t + 1 < T:
                    g2_cur = g2_next

            # tail: logits for the final token tile
            for vn in range(NVC):
                emit_logits_head(NT - 1, vn, [0, 1, 2, 3])
                emit_logits_tail(NT - 1, vn, [])

    nc.compile()
    _cached[key] = nc
    return nc


def _prep(features, captions, W_ih, W_hh, b_ih, b_hh, W_out, b_out, emb):
    features = np.asarray(features, dtype=np.float32)
    captions = np.asarray(captions)
    W_ih = np.asarray(W_ih, dtype=np.float32)
    W_hh = np.asarray(W_hh, dtype=np.float32)
    b_ih = np.asarray(b_ih, dtype=np.float32)
    b_hh = np.asarray(b_hh, dtype=np.float32)
    W_out = np.asarray(W_out, dtype=np.float32)
    b_out = np.asarray(b_out, dtype=np.float32)
    emb = np.asarray(emb, dtype=np.float32)

    # x sequence: t=0 is features, t>0 embeds caption[:, t]; t-major tokens
    xs = np.concatenate([features[:, None, :], emb[captions[:, 1:]]], axis=1)
    xs = np.ascontiguousarray(xs.transpose(1, 0, 2).reshape(T * B, E))
    xT = np.ascontiguousarray(xs.T).astype(bfnp)               # [E, T*B]

    # device gate column order: col = 512*jh + 128*c + hh  (c in f,g,i,o),
    # mapping to torch gate rows {i:0, f:512, g:1024, o:1536} + 128*jh + hh
    Wcat = np.concatenate([W_ih, W_hh], axis=1).copy()         # [2048, 1024]
    biasg_f = (b_ih + b_hh).copy()
    # g rows scaled by 2: tanh(z) = 2*sigmoid(2z) - 1
    Wcat[1024:1536] *= 2.0
    biasg_f[1024:1536] *= 2.0
    orig_base = np.array([512, 1024, 0, 1536])                 # f, g, i, o
    jh = np.arange(2048) // 512
    c = (np.arange(2048) % 512) // 128
    hh = np.arange(2048) % 128
    rows = orig_base[c] + 128 * jh + hh
    wt = np.ascontiguousarray(Wcat[rows].T).astype(bfnp)       # [1024, 2048]
    # bias tile [128, 512]: partition 32jh+b gets biasg row-block jh
    bg = biasg_f[rows].reshape(4, 512)
    biasg = np.ascontiguousarray(np.repeat(bg, B, axis=0)).astype(np.float32)

    ident = np.eye(P, dtype=bfnp)

    base = dict(xT=xT, wt=wt, biasg=biasg, ident=ident)
    in_maps = []
    for ci in range(NCORES):
        sl = slice(VC * ci, VC * (ci + 1))
        wot = np.ascontiguousarray(W_out[sl, :].T).astype(bfnp)      # [512, 4000]
        bout = np.ascontiguousarray(
            np.broadcast_to(b_out[sl][None, :], (P, VC))
        ).astype(np.float32)
        in_maps.append(dict(base, wot=wot, bout=bout))

    return in_maps


def build_in_maps(inputs):
    return _prep(**inputs)


def kernel(**inputs):
    in_maps = build_in_maps(inputs)
    nc = _build_nc()
    res = run_bass_kernel_spmd(nc, in_maps, core_ids=list(range(NCORES)))
    _cached["last_results"] = res

    # per-core out is [T*B, VC] t-major; reassemble to [B, T, V]
    outs = [
        r["out"].reshape(T, B, VC).swapaxes(0, 1) for r in res.results
    ]
    return np.ascontiguousarray(np.concatenate(outs, axis=2))
